# revision 1
# baseline (speedup 1.0000x reference)
"""Trainium2 Bass kernel for ClippingAttentionEngine.

Sharding: core c -> (batch b = c//2, head-group hg = c%2, 8 heads each).
Each core computes Q/K/V projections for its 8 heads, attention, and the
partial output projection over its head slice; host sums the two per-batch
partials (tensor-parallel over heads, per the sharding hint).

The per-batch sparse/dense branch is folded into a single dense-shaped
program: a host-built additive bias matrix B encodes either the dense
prior bias (0 / -lambda) or, for sparse batches, log(multiplicity) of each
key under prior_indices/prior_index_mask (-1e4 where never indexed), which
makes dense softmax(QK^T*scale + B) @ V exactly reproduce the gathered
sparse softmax (duplicates included).

Device pipeline per core (all matmuls float32r):
  A) xT/W streamed in; Q^T,K^T ([d',s] layout) and V (natural, with a
     ones-column per head for the softmax denominator) projected.
  B) per (head-pair, q-half) group: scores^T = B^T (via identity matmul)
     + K^T.T @ Q^T accumulated in PSUM, exp on ACT into P^T tiles; the
     attn@V' accumulation + normalization of the PREVIOUS group is
     software-pipelined against the current group's scores.
  C) partial out-projection from the normalized attn^T tiles.
"""

import sys

sys.path.insert(0, "/opt/trn_rl_repo")

import ml_dtypes
import numpy as np

import concourse.bass as bass
import concourse.tile as tile
from concourse import bacc, mybir
from concourse.alu_op_type import AluOpType
from concourse.bass_utils import run_bass_kernel_spmd

B, S, D, H = 4, 1024, 1024, 16
DH = D // H          # 64
HPC = 8              # heads per core
N_CORES = 8
KT = S // 128        # 8 k tiles
DCH = D // 128       # 8 contraction chunks
LAMBDA_MAX, ALPHA, SPARSE_THRESHOLD = 10.0, 5.0, 1.0

F32 = mybir.dt.float32
F32R = mybir.dt.float32r
BF16 = mybir.dt.bfloat16
EXP = mybir.ActivationFunctionType.Exp
IDENT = mybir.ActivationFunctionType.Identity


def build_program():
    nc = bacc.Bacc("TRN2", target_bir_lowering=False, debug=False,
                   num_devices=N_CORES)

    d_xt = nc.dram_tensor("xt", [D, S], F32R, kind="ExternalInput").ap()
    d_wqt = nc.dram_tensor("wqt", [D, 512], F32R, kind="ExternalInput").ap()
    d_wkt = nc.dram_tensor("wkt", [D, 512], F32R, kind="ExternalInput").ap()
    d_wvt = nc.dram_tensor("wvt", [D, 512], F32R, kind="ExternalInput").ap()
    d_wot = nc.dram_tensor("wot", [512, D], F32R, kind="ExternalInput").ap()
    d_bt = nc.dram_tensor("bt", [S, S], BF16, kind="ExternalInput").ap()
    d_bq = nc.dram_tensor("bq", [128, 4], F32, kind="ExternalInput").ap()
    d_bk = nc.dram_tensor("bk", [128, 4], F32, kind="ExternalInput").ap()
    d_bv = nc.dram_tensor("bv", [1, 512], F32R, kind="ExternalInput").ap()
    d_bo = nc.dram_tensor("bo", [1, D], F32R, kind="ExternalInput").ap()
    d_id = nc.dram_tensor("ident", [128, 128], BF16, kind="ExternalInput").ap()
    d_ones = nc.dram_tensor("ones", [128, 512], F32R, kind="ExternalInput").ap()
    d_out = nc.dram_tensor("out", [S, D], F32, kind="ExternalOutput").ap()

    with tile.TileContext(nc) as tc:
        with (
            tc.tile_pool(name="const", bufs=1) as constp,
            tc.tile_pool(name="main", bufs=1) as mainp,
        ):
            ident = constp.tile([128, 128], BF16, tag="ident")
            nc.sync.dma_start(ident[:], d_id[:])
            ones = constp.tile([1, 512], F32R, tag="ones")
            nc.sync.dma_start(ones[:], d_ones[0:1, :])
            onecol = constp.tile([128, 8], F32R, tag="onecol")
            nc.sync.dma_start(onecol[:], d_ones[:, 0:8])
            bq_sb = constp.tile([128, 4], F32, tag="bq")
            nc.sync.dma_start(bq_sb[:], d_bq[:])
            bk_sb = constp.tile([128, 4], F32, tag="bk")
            nc.sync.dma_start(bk_sb[:], d_bk[:])
            bv_sb = constp.tile([1, 512], F32R, tag="bv")
            nc.sync.dma_start(bv_sb[:], d_bv[:])
            bo_sb = constp.tile([1, D], F32R, tag="bo")
            nc.sync.dma_start(bo_sb[:], d_bo[:])

            # Persistent arrays.
            qt_sb = [mainp.tile([128, S], F32R, tag=f"qt{m}", name=f"qt{m}")
                     for m in range(4)]
            kt_sb = [mainp.tile([128, S], F32R, tag=f"kt{m}", name=f"kt{m}")
                     for m in range(4)]
            vp_sb = [mainp.tile([128, HPC * (DH + 1)], F32R, tag=f"vp{sb}",
                                name=f"vp{sb}") for sb in range(8)]
            at_sb = [mainp.tile([128, S], F32R, tag=f"at{m}", name=f"at{m}")
                     for m in range(4)]
            wot_sb = [mainp.tile([128, D], F32R, tag=f"wot{mc}", name=f"wot{mc}")
                      for mc in range(4)]
            bt_sb = [mainp.tile([128, S], BF16, tag=f"bt{k}", name=f"bt{k}")
                     for k in range(KT)]
            # ---- Stage A: projections (scoped: xT + W slices + wide psum) ----
            with (
                tc.tile_pool(name="stageA", bufs=1) as pA,
                tc.tile_pool(name="ppp", bufs=2, space="PSUM") as ppp,
            ):
                xt_sb = [pA.tile([128, S], F32R, tag=f"xt{c}", name=f"xt{c}")
                         for c in range(DCH)]
                for c in range(DCH):
                    nc.sync.dma_start(xt_sb[c][:], d_xt[c * 128:(c + 1) * 128, :])
                w_sb = {}
                for nm, dap in (("q", d_wqt), ("k", d_wkt), ("v", d_wvt)):
                    w_sb[nm] = [pA.tile([128, 512], F32R, tag=f"w{nm}{c}",
                                        name=f"w{nm}{c}") for c in range(DCH)]
                    for c in range(DCH):
                        nc.sync.dma_start(w_sb[nm][c][:],
                                          dap[c * 128:(c + 1) * 128, :])

                # Q^T / K^T: psum[d'128, s1024] = sum_c W^T[c][:,d'].T @ xT[c]
                for nm, dst, bias in (("q", qt_sb, bq_sb), ("k", kt_sb, bk_sb)):
                    for m in range(4):
                        pp = ppp.tile([128, 1024], F32, tag="pp")
                        for st in range(2):
                            for c in range(DCH):
                                nc.tensor.matmul(
                                    pp[:, st * 512:(st + 1) * 512],
                                    w_sb[nm][c][:, m * 128:(m + 1) * 128],
                                    xt_sb[c][:, st * 512:(st + 1) * 512],
                                    start=(c == 0), stop=(c == DCH - 1))
                        nc.scalar.activation(dst[m][:], pp[:],
                                             IDENT, bias=bias[:, m:m + 1])

                # V natural: psum[s128, dh512] = sum_c xT[c][:,sblk].T @ WvT[c]
                for sb in range(8):
                    pp = ppp.tile([128, 1024], F32, tag="pp")
                    ps = pp[:, 0:512]
                    for c in range(DCH):
                        nc.tensor.matmul(
                            ps,
                            xt_sb[c][:, sb * 128:(sb + 1) * 128],
                            w_sb["v"][c][:],
                            start=(c == 0), stop=False)
                    nc.tensor.matmul(ps, ones[0:1, 0:128], bv_sb[:],
                                     start=False, stop=True)
                    vp3 = vp_sb[sb].rearrange("p (h d) -> p h d", d=DH + 1)
                    nc.vector.tensor_copy(
                        vp3[:, :, 0:DH],
                        ps.rearrange("p (h d) -> p h d", d=DH))
                    nc.vector.tensor_copy(
                        vp3[:, :, DH:DH + 1],
                        onecol[:].rearrange("p (h o) -> p h o", o=1))

            for k in range(KT):
                nc.sync.dma_start(bt_sb[k][:], d_bt[k * 128:(k + 1) * 128, :])
            for mc in range(4):
                nc.sync.dma_start(wot_sb[mc][:], d_wot[mc * 128:(mc + 1) * 128, :])

            # ---- Stage B: attention, software-pipelined by (pair, q-half) --
            with (
                tc.tile_pool(name="ptp", bufs=16) as ptp,
                tc.tile_pool(name="smallp", bufs=2) as smallp,
                tc.tile_pool(name="outp", bufs=2) as outp,
                tc.tile_pool(name="psS", bufs=2, space="PSUM") as psS,
                tc.tile_pool(name="psO", bufs=4, space="PSUM") as psO,
            ):
                def emit_scores(m, q):
                    pts = {}
                    for k in range(KT):
                        ps = psS.tile([128, 1024], F32, tag="ps")
                        for hh in range(2):
                            nc.tensor.matmul(
                                ps[:, hh * 512:(hh + 1) * 512], ident[:],
                                bt_sb[k][:, q * 512:(q + 1) * 512],
                                start=True, stop=False)
                        for hh in range(2):
                            nc.tensor.matmul(
                                ps[:, hh * 512:(hh + 1) * 512],
                                kt_sb[m][hh * 64:(hh + 1) * 64,
                                         k * 128:(k + 1) * 128],
                                qt_sb[m][hh * 64:(hh + 1) * 64,
                                         q * 512:(q + 1) * 512],
                                start=False, stop=True,
                                tile_position=(hh * 64, 0))
                        pt = ptp.tile([128, 1024], F32R, tag="pt")
                        nc.scalar.activation(pt[:], ps[:], EXP)
                        pts[k] = pt
                    return pts

                def emit_attnv(m, q, pts):
                    pos = []
                    for hh in range(2):
                        h = m * 2 + hh
                        po = psO.tile([DH + 1, 512], F32, tag="att",
                                      name=f"po{hh}")
                        for k in range(KT):
                            nc.tensor.matmul(
                                po[:],
                                vp_sb[k][:, h * (DH + 1):(h + 1) * (DH + 1)],
                                pts[k][:, hh * 512:(hh + 1) * 512],
                                start=(k == 0), stop=(k == KT - 1))
                        pos.append(po)
                    for hh in range(2):
                        zrow = smallp.tile([1, 512], F32, tag="zrow",
                                           name=f"zr{hh}")
                        nc.vector.tensor_copy(zrow[:], pos[hh][DH:DH + 1, :])
                        rec = smallp.tile([1, 512], F32, tag="rec",
                                          name=f"rc{hh}")
                        scr = smallp.tile([1, 512], F32, tag="scr",
                                          name=f"sc{hh}")
                        nc.vector.reciprocal_approx_accurate(rec[:], zrow[:],
                                                             scr[:])
                        bc = smallp.tile([64, 512], F32, tag="bc",
                                         name=f"bc{hh}")
                        nc.gpsimd.partition_broadcast(bc[:], rec[:])
                        nc.vector.tensor_tensor(
                            at_sb[m][hh * 64:(hh + 1) * 64,
                                     q * 512:(q + 1) * 512],
                            pos[hh][0:DH, :], bc[:], AluOpType.mult)

                groups = [(m, q) for m in range(4) for q in range(2)]
                prev = None
                for g in groups:
                    pts = emit_scores(*g)
                    if prev is not None:
                        emit_attnv(prev[0][0], prev[0][1], prev[1])
                    prev = (g, pts)
                emit_attnv(prev[0][0], prev[0][1], prev[1])

                # ---- Stage C: partial output projection ----
                for sb in range(8):
                    ot = outp.tile([128, D], F32, tag="ot")
                    for q in range(2):
                        ps = psS.tile([128, 512], F32, tag="ps")
                        for mc in range(4):
                            nc.tensor.matmul(
                                ps[:],
                                at_sb[mc][:, sb * 128:(sb + 1) * 128],
                                wot_sb[mc][:, q * 512:(q + 1) * 512],
                                start=(mc == 0), stop=False)
                        nc.tensor.matmul(ps[:], ones[0:1, 0:128],
                                         bo_sb[0:1, q * 512:(q + 1) * 512],
                                         start=False, stop=True)
                        nc.scalar.copy(ot[:, q * 512:(q + 1) * 512], ps[:])
                    nc.sync.dma_start(d_out[sb * 128:(sb + 1) * 128, :], ot[:])

    nc.compile()
    return nc


_prog = None


def _get_prog():
    global _prog
    if _prog is None:
        _prog = build_program()
    return _prog


def _host_prep(x, prior_mask, prior_indices, prior_index_mask, u_prev,
               Wq, bq, Wk, bk, Wv, bv, Wo, bo):
    f32 = np.float32
    x = np.asarray(x, f32)
    pm = np.asarray(prior_mask, bool)
    idx = np.asarray(prior_indices)
    pim = np.asarray(prior_index_mask, bool)
    u = np.asarray(u_prev, f32).reshape(B)
    Wq, Wk, Wv, Wo = (np.asarray(w, f32) for w in (Wq, Wk, Wv, Wo))
    bq, bk, bv, bo = (np.asarray(v, f32) for v in (bq, bk, bv, bo))

    scale = f32(1.0 / np.sqrt(DH))
    lam = (LAMBDA_MAX * np.exp(-ALPHA * u.astype(np.float64))).astype(f32)
    use_sparse = lam >= SPARSE_THRESHOLD

    # Sparse multiplicity bias (shared across batches): log(count) or -1e4.
    bts_sparse = None
    if use_sparse.any():
        cnt = np.zeros((S, S + 1), np.int32)
        np.add.at(cnt, (np.arange(S)[:, None],
                        np.where(pim, idx, S).astype(np.int64)), 1)
        cnt = cnt[:, :S]
        bsp = np.where(cnt > 0, np.log(np.maximum(cnt, 1)).astype(f32),
                       f32(-10000.0))
        bts_sparse = np.ascontiguousarray(bsp.T.astype(ml_dtypes.bfloat16))

    bts = []
    for b in range(B):
        if use_sparse[b]:
            bts.append(bts_sparse)
        else:
            bd = np.where(pm, f32(0.0), f32(-lam[b]))
            bts.append(np.ascontiguousarray(bd.T.astype(ml_dtypes.bfloat16)))

    in_maps = []
    for c in range(N_CORES):
        b = c // 2
        hg = c % 2
        hsl = slice(hg * 512, (hg + 1) * 512)
        in_maps.append({
            "xt": np.ascontiguousarray(x[b].T),
            "wqt": np.ascontiguousarray((Wq[hsl] * scale).T),
            "wkt": np.ascontiguousarray(Wk[hsl].T),
            "wvt": np.ascontiguousarray(Wv[hsl].T),
            "wot": np.ascontiguousarray(Wo[:, hsl].T),
            "bt": bts[b],
            "bq": np.ascontiguousarray((bq[hsl] * scale).reshape(4, 128).T),
            "bk": np.ascontiguousarray(bk[hsl].reshape(4, 128).T),
            "bv": np.ascontiguousarray(bv[hsl].reshape(1, 512)),
            "bo": np.ascontiguousarray((bo * f32(0.5)).reshape(1, D)),
            "ident": np.eye(128, dtype=ml_dtypes.bfloat16),
            "ones": np.ones((128, 512), dtype=f32),
        })
    return in_maps


def kernel(**inputs):
    in_maps = _host_prep(**inputs)
    nc = _get_prog()
    res = run_bass_kernel_spmd(nc, in_maps, core_ids=list(range(N_CORES)))
    out = np.empty((B, S, D), np.float32)
    for b in range(B):
        out[b] = res.results[2 * b]["out"] + res.results[2 * b + 1]["out"]
    return out



# revision 5
# speedup vs baseline: 1.2278x; 1.2278x over previous
"""Trainium2 Bass kernel for ClippingAttentionEngine.

Sharding: core c -> (batch b = c//2, head-group hg = c%2, 8 heads each).
Each core computes Q/K/V projections for its 8 heads, attention, and the
partial transposed output projection over its head slice; host sums the two
per-batch partials, transposes, and adds the constant bias terms
(bo + bv @ Wo^T -- the V bias passes through softmax averaging unchanged).

The per-sample sparse/dense branch is folded into a single dense-shaped
program via a MULTIPLICATIVE prior M (host-built, bf16):
  dense batch:  M[q,k] = pm[q,k] ? 1 : exp(-lambda)
  sparse batch: M[q,k] = multiplicity of key k in prior_indices[q] (masked
                slots excluded), so P = exp(s) * M reproduces the gathered
                sparse softmax exactly (duplicates included, 0 = exact mask).

All matmuls are bf16 (fp32 PSUM accumulate). Engine assignment:
  PE   : projections, scores, attn@V, out-projection (no bias/identity mms)
  ACT  : exp only (exp table stays loaded, no table thrash)
  DVE  : Q/K PSUM->SBUF copies w/ bias, V copies, P = exp(s) * M multiply
         (4x bf16 mode), softmax reciprocal + normalize multiplies
  Pool : ones-column memsets, z broadcast across partitions
Stage C accumulates out^T in PSUM and DMAs PSUM->DRAM directly.
"""

import sys

sys.path.insert(0, "/opt/trn_rl_repo")

import ml_dtypes
import numpy as np

import concourse.bass as bass
import concourse.tile as tile
from concourse import bacc, mybir
from concourse.alu_op_type import AluOpType
from concourse.bass_utils import run_bass_kernel_spmd

B, S, D, H = 4, 1024, 1024, 16
DH = D // H          # 64
HPC = 8              # heads per core
N_CORES = 8
KT = S // 128        # 8 k tiles
DCH = D // 128       # 8 contraction chunks
LAMBDA_MAX, ALPHA, SPARSE_THRESHOLD = 10.0, 5.0, 1.0

F32 = mybir.dt.float32
BF16 = mybir.dt.bfloat16
EXP = mybir.ActivationFunctionType.Exp


def build_program():
    nc = bacc.Bacc("TRN2", target_bir_lowering=False, debug=False,
                   num_devices=N_CORES)

    d_xt = nc.dram_tensor("xt", [D, S], BF16, kind="ExternalInput").ap()
    d_wqt = nc.dram_tensor("wqt", [D, 512], BF16, kind="ExternalInput").ap()
    d_wkt = nc.dram_tensor("wkt", [D, 512], BF16, kind="ExternalInput").ap()
    d_wvt = nc.dram_tensor("wvt", [D, 512], BF16, kind="ExternalInput").ap()
    d_wot = nc.dram_tensor("wot", [512, D], BF16, kind="ExternalInput").ap()
    d_mtd = nc.dram_tensor("mtd", [S, 2048], BF16, kind="ExternalInput").ap()
    d_bq = nc.dram_tensor("bq", [128, 4], F32, kind="ExternalInput").ap()
    d_bk = nc.dram_tensor("bk", [128, 4], F32, kind="ExternalInput").ap()
    d_out = nc.dram_tensor("out", [D, S], F32, kind="ExternalOutput").ap()

    with tile.TileContext(nc) as tc:
        with (
            tc.tile_pool(name="const", bufs=1) as constp,
            tc.tile_pool(name="main", bufs=1) as mainp,
            tc.tile_pool(name="inp", bufs=1) as inp,
            tc.tile_pool(name="ptp", bufs=16) as ptp,
            tc.tile_pool(name="smallp", bufs=4) as smallp,
            tc.tile_pool(name="psQK", bufs=1, space="PSUM") as psQK,
            tc.tile_pool(name="psS", bufs=2, space="PSUM") as psS,
            tc.tile_pool(name="psV", bufs=2, space="PSUM") as psV,
        ):
            bq_sb = constp.tile([128, 4], F32, tag="bq")
            nc.sync.dma_start(bq_sb[:], d_bq[:])
            bk_sb = constp.tile([128, 4], F32, tag="bk")
            nc.sync.dma_start(bk_sb[:], d_bk[:])

            # Persistent arrays.
            qt_sb = [mainp.tile([128, S], BF16, tag=f"qt{m}", name=f"qt{m}")
                     for m in range(4)]
            kt_sb = [mainp.tile([128, S], BF16, tag=f"kt{m}", name=f"kt{m}")
                     for m in range(4)]
            vp_sb = [mainp.tile([128, HPC * (DH + 1)], BF16, tag=f"vp{sb}",
                                name=f"vp{sb}") for sb in range(8)]
            at_sb = [mainp.tile([128, S], BF16, tag=f"at{m}", name=f"at{m}")
                     for m in range(4)]
            wot_sb = [mainp.tile([128, D], BF16, tag=f"wot{mc}",
                                 name=f"wot{mc}") for mc in range(4)]
            mtd_sb = [mainp.tile([128, 2048], BF16, tag=f"mtd{k}",
                                 name=f"mtd{k}") for k in range(KT)]

            # Stage-A inputs (persist until qk(m3)/V emitted).
            xt_sb = [inp.tile([128, S], BF16, tag=f"xt{c}", name=f"xt{c}")
                     for c in range(DCH)]
            w_sb = {nm: [inp.tile([128, 512], BF16, tag=f"w{nm}{c}",
                                  name=f"w{nm}{c}") for c in range(DCH)]
                    for nm in ("q", "k", "v")}
            # DMA order = need order: x/wq/wk chunks first (qk(m0)), then
            # M tiles (scores(0,0)), then wv (V), then wot (stage C).
            for c in range(DCH):
                nc.sync.dma_start(xt_sb[c][:], d_xt[c * 128:(c + 1) * 128, :])
                nc.sync.dma_start(w_sb["q"][c][:],
                                  d_wqt[c * 128:(c + 1) * 128, :])
                nc.sync.dma_start(w_sb["k"][c][:],
                                  d_wkt[c * 128:(c + 1) * 128, :])
            for k in range(KT):
                nc.sync.dma_start(mtd_sb[k][:], d_mtd[k * 128:(k + 1) * 128, :])
            for c in range(DCH):
                nc.sync.dma_start(w_sb["v"][c][:],
                                  d_wvt[c * 128:(c + 1) * 128, :])
            for mc in range(4):
                nc.sync.dma_start(wot_sb[mc][:],
                                  d_wot[mc * 128:(mc + 1) * 128, :])

            # Ones columns of vp (softmax denominator rows): set once.
            for sb in range(8):
                vp3 = vp_sb[sb].rearrange("p (h d) -> p h d", d=DH + 1)
                nc.gpsimd.memset(vp3[:, :, DH:DH + 1], 1.0)

            # ---- emission helpers ----
            def emit_qk(m):
                """Q^T,K^T head-pair m: psum[d'128, s1024]; DVE copy+bias."""
                for nm, dst, bias in (("q", qt_sb, bq_sb), ("k", kt_sb, bk_sb)):
                    pp = psQK.tile([128, 1024], F32, tag="pp")
                    for st in range(2):
                        for c in range(DCH):
                            nc.tensor.matmul(
                                pp[:, st * 512:(st + 1) * 512],
                                w_sb[nm][c][:, m * 128:(m + 1) * 128],
                                xt_sb[c][:, st * 512:(st + 1) * 512],
                                start=(c == 0), stop=(c == DCH - 1))
                    nc.vector.tensor_scalar_add(dst[m][:], pp[:],
                                                bias[:, m:m + 1])

            def emit_v(sb):
                """V block sb: psum[s128, dh512] -> vp (strided bf16 copy)."""
                pv = psV.tile([128, 512], F32, tag="pv")
                for c in range(DCH):
                    nc.tensor.matmul(
                        pv[:],
                        xt_sb[c][:, sb * 128:(sb + 1) * 128],
                        w_sb["v"][c][:],
                        start=(c == 0), stop=(c == DCH - 1))
                vp3 = vp_sb[sb].rearrange("p (h d) -> p h d", d=DH + 1)
                nc.vector.tensor_copy(
                    vp3[:, :, 0:DH],
                    pv[:].rearrange("p (h d) -> p h d", d=DH))

            def emit_scores(m, q):
                """Scores k-tiles for group (m,q): P^T = exp(K^T.T@Q^T) * M."""
                pts = {}
                for k in range(KT):
                    ps = psS.tile([128, 1024], F32, tag="ps")
                    for hh in range(2):
                        nc.tensor.matmul(
                            ps[:, hh * 512:(hh + 1) * 512],
                            kt_sb[m][hh * 64:(hh + 1) * 64,
                                     k * 128:(k + 1) * 128],
                            qt_sb[m][hh * 64:(hh + 1) * 64,
                                     q * 512:(q + 1) * 512],
                            start=True, stop=True,
                            tile_position=(hh * 64, 0))
                    pt = ptp.tile([128, 1024], BF16, tag="pt")
                    nc.scalar.activation(pt[:], ps[:], EXP)
                    nc.vector.tensor_tensor(
                        pt[:], pt[:], mtd_sb[k][:, q * 1024:(q + 1) * 1024],
                        AluOpType.mult)
                    pts[k] = pt
                return pts

            def emit_attnv(m, q, pts):
                """attn@V' for group (m,q) + normalize into at_sb (bf16)."""
                pos = []
                for hh in range(2):
                    h = m * 2 + hh
                    po = psV.tile([DH + 1, 512], F32, tag="pv",
                                  name=f"po{hh}")
                    for k in range(KT):
                        nc.tensor.matmul(
                            po[:],
                            vp_sb[k][:, h * (DH + 1):(h + 1) * (DH + 1)],
                            pts[k][:, hh * 512:(hh + 1) * 512],
                            start=(k == 0), stop=(k == KT - 1))
                    pos.append(po)
                for hh in range(2):
                    # reciprocal_approx_fast mis-addresses partition-offset
                    # PSUM inputs; stage the z row at partition 0 in SBUF.
                    zrow = smallp.tile([1, 512], F32, tag="zrow",
                                       name=f"zr{hh}")
                    nc.vector.tensor_copy(zrow[:], pos[hh][DH:DH + 1, :])
                    rec = smallp.tile([1, 512], F32, tag="rec",
                                      name=f"rc{hh}")
                    nc.vector.reciprocal_approx_fast(rec[:], zrow[:])
                    bc = smallp.tile([64, 512], F32, tag="bc",
                                     name=f"bc{hh}")
                    nc.gpsimd.partition_broadcast(bc[:], rec[:])
                    nc.vector.tensor_tensor(
                        at_sb[m][hh * 64:(hh + 1) * 64,
                                 q * 512:(q + 1) * 512],
                        pos[hh][0:DH, :], bc[:], AluOpType.mult)

            def emit_outproj(dt, q):
                """out^T tile: psum[d_out 128, s 512] -> SBUF -> DMA out."""
                pc = psS.tile([128, 1024], F32, tag="ps", name=f"pc{dt}")
                pcs = pc[:, 0:512]
                for mc in range(4):
                    nc.tensor.matmul(
                        pcs,
                        wot_sb[mc][:, dt * 128:(dt + 1) * 128],
                        at_sb[mc][:, q * 512:(q + 1) * 512],
                        start=(mc == 0), stop=(mc == 3))
                ot = smallp.tile([128, 512], F32, tag="ot", name=f"ot{dt}")
                nc.scalar.copy(ot[:], pcs)
                nc.sync.dma_start(
                    d_out[dt * 128:(dt + 1) * 128, q * 512:(q + 1) * 512],
                    ot[:])

            # ---- static schedule ----
            # Prologue: heads m0, first scores early so ACT starts streaming,
            # then V / remaining projections fill PE while ACT works.
            emit_qk(0)
            sc = emit_scores(0, 0)
            for sb in range(8):
                emit_v(sb)
            emit_qk(1)
            prev = ((0, 0), sc)

            # Steady state: groups q-major; AV(g-1) + filler behind S(g).
            order = [(1, 0), (2, 0), (3, 0), (0, 1), (1, 1), (2, 1), (3, 1)]
            fillers = {(1, 0): lambda: emit_qk(2), (2, 0): lambda: emit_qk(3)}
            # C(q0) tiles interleave into the q1 groups: at iteration g the
            # emit_attnv(prev) call has just retired AV(3,0), so all at[*][:,
            # 0:512] slices are written before any C(*,0) read.
            outq0 = [(0, 1), (1, 1), (2, 1), (3, 1)]
            for g in order:
                sc = emit_scores(*g)
                emit_attnv(prev[0][0], prev[0][1], prev[1])
                if g in fillers:
                    fillers[g]()
                if g in outq0:
                    i = outq0.index(g)
                    emit_outproj(2 * i, 0)
                    emit_outproj(2 * i + 1, 0)
                prev = (g, sc)
            emit_attnv(prev[0][0], prev[0][1], prev[1])
            for dt in range(8):
                emit_outproj(dt, 1)

    nc.compile()
    return nc


_prog = None


def _get_prog():
    global _prog
    if _prog is None:
        _prog = build_program()
    return _prog


def _host_prep(x, prior_mask, prior_indices, prior_index_mask, u_prev,
               Wq, bq, Wk, bk, Wv, bv, Wo, bo):
    f32 = np.float32
    bf16 = ml_dtypes.bfloat16
    x = np.asarray(x, f32)
    pm = np.asarray(prior_mask, bool)
    idx = np.asarray(prior_indices)
    pim = np.asarray(prior_index_mask, bool)
    u = np.asarray(u_prev, f32).reshape(B)
    Wq, Wk, Wv, Wo = (np.asarray(w, f32) for w in (Wq, Wk, Wv, Wo))
    bq, bk, bv, bo = (np.asarray(v, f32) for v in (bq, bk, bv, bo))

    scale = f32(1.0 / np.sqrt(DH))
    lam = (LAMBDA_MAX * np.exp(-ALPHA * u.astype(np.float64))).astype(f32)
    use_sparse = lam >= SPARSE_THRESHOLD

    # Multiplicative prior M^T [k, q], duplicated per q-half for the
    # [hh0|hh1] P-tile layout: mtd[:, q*1024:(q+1)*1024] = [Mt_q | Mt_q].
    mtd_sparse = None
    if use_sparse.any():
        cnt = np.zeros((S, S + 1), f32)
        np.add.at(cnt, (np.arange(S)[:, None],
                        np.where(pim, idx, S).astype(np.int64)), 1.0)
        mt = np.ascontiguousarray(cnt[:, :S].T)
        mtd_sparse = np.concatenate(
            [mt[:, :512], mt[:, :512], mt[:, 512:], mt[:, 512:]],
            axis=1).astype(bf16)

    mtds = []
    for b in range(B):
        if use_sparse[b]:
            mtds.append(mtd_sparse)
        else:
            mt = np.where(pm, f32(1.0), np.exp(-lam[b], dtype=f32)).T
            mtds.append(np.concatenate(
                [mt[:, :512], mt[:, :512], mt[:, 512:], mt[:, 512:]],
                axis=1).astype(bf16))

    in_maps = []
    for c in range(N_CORES):
        b = c // 2
        hg = c % 2
        hsl = slice(hg * 512, (hg + 1) * 512)
        in_maps.append({
            "xt": np.ascontiguousarray(x[b].T).astype(bf16),
            "wqt": np.ascontiguousarray((Wq[hsl] * scale).T).astype(bf16),
            "wkt": np.ascontiguousarray(Wk[hsl].T).astype(bf16),
            "wvt": np.ascontiguousarray(Wv[hsl].T).astype(bf16),
            "wot": np.ascontiguousarray(Wo[:, hsl].T).astype(bf16),
            "mtd": mtds[b],
            "bq": np.ascontiguousarray((bq[hsl] * scale).reshape(4, 128).T),
            "bk": np.ascontiguousarray(bk[hsl].reshape(4, 128).T),
        })
    return in_maps


def kernel(**inputs):
    in_maps = _host_prep(**inputs)
    nc = _get_prog()
    res = run_bass_kernel_spmd(nc, in_maps, core_ids=list(range(N_CORES)))
    # Constant bias terms pass through the attention average unchanged:
    # out += bo + bv @ Wo^T  (z-normalized ones-column makes bv exact).
    bv = np.asarray(inputs["bv"], np.float32)
    bo = np.asarray(inputs["bo"], np.float32)
    Wo = np.asarray(inputs["Wo"], np.float32)
    const_row = bo + bv @ Wo.T
    out = np.empty((B, S, D), np.float32)
    for b in range(B):
        pt = res.results[2 * b]["out"] + res.results[2 * b + 1]["out"]
        out[b] = pt.T + const_row
    return out


# revision 8
# speedup vs baseline: 1.2344x; 1.0054x over previous
"""Trainium2 Bass kernel for ClippingAttentionEngine.

Sharding: core c -> (batch b = c//2, head-group hg = c%2, 8 heads each).
Each core computes Q/K/V projections for its 8 heads, attention, and the
partial transposed output projection over its head slice; host sums the two
per-batch partials, transposes, and adds the constant bias terms
(bo + bv @ Wo^T -- the V bias passes through softmax averaging unchanged).

The per-sample sparse/dense branch is folded into a single dense-shaped
program via a MULTIPLICATIVE prior M (host-built, bf16):
  dense batch:  M[q,k] = pm[q,k] ? 1 : exp(-lambda)
  sparse batch: M[q,k] = multiplicity of key k in prior_indices[q] (masked
                slots excluded), so P = exp(s) * M reproduces the gathered
                sparse softmax exactly (duplicates included, 0 = exact mask).

All matmuls are bf16 (fp32 PSUM accumulate). Engine assignment:
  PE   : projections, scores (row-tiled hh pairs run concurrently),
         attn@V' (ones-column gives the softmax denominator), out-proj
  ACT  : exp (exp table stays loaded) + stage-C PSUM->SBUF copies
  DVE  : Q/K PSUM->SBUF copies w/ bias, P = exp(s)*M multiply (2x bf16),
         z-row staging + reciprocal
  Pool : softmax normalize multiplies, V copies, z broadcast, memsets
Inputs arrive as a few large merged DMAs split across both HWDGE rings
(sync + scalar) to cut ring serialization; outputs alternate rings.
"""

import sys

sys.path.insert(0, "/opt/trn_rl_repo")

import ml_dtypes
import numpy as np

import concourse.bass as bass
import concourse.tile as tile
from concourse import bacc, mybir
from concourse.alu_op_type import AluOpType
from concourse.bass_utils import run_bass_kernel_spmd

B, S, D, H = 4, 1024, 1024, 16
DH = D // H          # 64
HPC = 8              # heads per core
N_CORES = 8
KT = S // 128        # 8 k tiles
DCH = D // 128       # 8 contraction chunks
LAMBDA_MAX, ALPHA, SPARSE_THRESHOLD = 10.0, 5.0, 1.0

F32 = mybir.dt.float32
BF16 = mybir.dt.bfloat16
EXP = mybir.ActivationFunctionType.Exp


def build_program():
    nc = bacc.Bacc("TRN2", target_bir_lowering=False, debug=False,
                   num_devices=N_CORES)

    d_xt = nc.dram_tensor("xt", [D, S], BF16, kind="ExternalInput").ap()
    d_wqt = nc.dram_tensor("wqt", [D, 512], BF16, kind="ExternalInput").ap()
    d_wkt = nc.dram_tensor("wkt", [D, 512], BF16, kind="ExternalInput").ap()
    d_wvt = nc.dram_tensor("wvt", [D, 512], BF16, kind="ExternalInput").ap()
    d_wot = nc.dram_tensor("wot", [512, D], BF16, kind="ExternalInput").ap()
    d_mtd = nc.dram_tensor("mtd", [S, 2048], BF16, kind="ExternalInput").ap()
    d_bq = nc.dram_tensor("bq", [128, 4], F32, kind="ExternalInput").ap()
    d_bk = nc.dram_tensor("bk", [128, 4], F32, kind="ExternalInput").ap()
    d_out = nc.dram_tensor("out", [D, S], F32, kind="ExternalOutput").ap()

    with tile.TileContext(nc) as tc:
        with (
            tc.tile_pool(name="const", bufs=1) as constp,
            tc.tile_pool(name="main", bufs=1) as mainp,
            tc.tile_pool(name="inp", bufs=1) as inp,
            tc.tile_pool(name="ptp", bufs=16) as ptp,
            tc.tile_pool(name="smallp", bufs=4) as smallp,
            tc.tile_pool(name="psS", bufs=2, space="PSUM") as psS,
            tc.tile_pool(name="psV", bufs=4, space="PSUM") as psV,
        ):
            bq_sb = constp.tile([128, 4], F32, tag="bq")
            nc.scalar.dma_start(bq_sb[:], d_bq[:])
            bk_sb = constp.tile([128, 4], F32, tag="bk")
            nc.scalar.dma_start(bk_sb[:], d_bk[:])

            # Persistent arrays.
            qt_sb = [mainp.tile([128, S], BF16, tag=f"qt{m}", name=f"qt{m}")
                     for m in range(4)]
            kt_sb = [mainp.tile([128, S], BF16, tag=f"kt{m}", name=f"kt{m}")
                     for m in range(4)]
            vp_sb = [mainp.tile([128, HPC * (DH + 1)], BF16, tag=f"vp{sb}",
                                name=f"vp{sb}") for sb in range(8)]
            at_sb = [mainp.tile([128, S], BF16, tag=f"at{m}", name=f"at{m}")
                     for m in range(4)]
            wot_sb = mainp.tile([128, 4 * D], BF16, tag="wot", name="wot")
            mtd_sb = mainp.tile([128, KT * 2048], BF16, tag="mtd", name="mtd")

            # Stage-A inputs, merged for large DMAs.
            xt_sb = inp.tile([128, DCH * S], BF16, tag="xt", name="xt")
            w_sb = {nm: inp.tile([128, DCH * 512], BF16, tag=f"w{nm}",
                                 name=f"w{nm}") for nm in ("q", "k", "v")}

            def chunked(dst, src, n, csz, ring, pieces):
                """DMA dram [n*128, csz] -> sbuf [128, n*csz] in `pieces`."""
                d3 = dst.rearrange("p (c s) -> p c s", s=csz)
                s3 = src.rearrange("(c p) s -> p c s", p=128)
                step = n // pieces
                for i in range(pieces):
                    sl = slice(i * step, (i + 1) * step)
                    ring.dma_start(d3[:, sl], s3[:, sl])

            # sync ring: xt, wq, wk (needed first, in that order).
            chunked(xt_sb, d_xt, DCH, S, nc.sync, 4)
            chunked(w_sb["q"], d_wqt, DCH, 512, nc.sync, 2)
            chunked(w_sb["k"], d_wkt, DCH, 512, nc.sync, 2)
            # scalar ring: mtd (needed by first mult), wv, wot.
            chunked(mtd_sb, d_mtd, KT, 2048, nc.scalar, 4)
            chunked(w_sb["v"], d_wvt, DCH, 512, nc.scalar, 2)
            chunked(wot_sb, d_wot, 4, D, nc.scalar, 1)

            # Ones columns of vp (softmax denominator rows): set once.
            for sb in range(8):
                vp3 = vp_sb[sb].rearrange("p (h d) -> p h d", d=DH + 1)
                nc.gpsimd.memset(vp3[:, :, DH:DH + 1], 1.0)

            # ---- emission helpers ----
            def emit_qk(m):
                """Q^T,K^T head-pair m: psum[d'128, s1024]; DVE copy+bias."""
                for nm, dst, bias in (("q", qt_sb, bq_sb), ("k", kt_sb, bk_sb)):
                    pp = psS.tile([128, 1024], F32, tag="ps", name=f"pp{nm}")
                    for st in range(2):
                        for c in range(DCH):
                            nc.tensor.matmul(
                                pp[:, st * 512:(st + 1) * 512],
                                w_sb[nm][:, c * 512 + m * 128:
                                         c * 512 + (m + 1) * 128],
                                xt_sb[:, c * S + st * 512:
                                      c * S + (st + 1) * 512],
                                start=(c == 0), stop=(c == DCH - 1))
                    nc.vector.tensor_scalar_add(dst[m][:], pp[:],
                                                bias[:, m:m + 1])

            def emit_v(sb):
                """V block sb: psum[s128, dh512] -> vp (strided bf16 copy)."""
                pv = psV.tile([128, 512], F32, tag="pv", name="pv")
                for c in range(DCH):
                    nc.tensor.matmul(
                        pv[:],
                        xt_sb[:, c * S + sb * 128:c * S + (sb + 1) * 128],
                        w_sb["v"][:, c * 512:(c + 1) * 512],
                        start=(c == 0), stop=(c == DCH - 1))
                vp3 = vp_sb[sb].rearrange("p (h d) -> p h d", d=DH + 1)
                nc.vector.tensor_copy(
                    vp3[:, :, 0:DH],
                    pv[:].rearrange("p (h d) -> p h d", d=DH))

            def emit_scores(m, q):
                """Scores k-tiles for group (m,q): P^T = exp(K^T.T@Q^T) * M."""
                pts = {}
                for k in range(KT):
                    ps = psS.tile([128, 1024], F32, tag="ps", name="ps")
                    for hh in range(2):
                        nc.tensor.matmul(
                            ps[:, hh * 512:(hh + 1) * 512],
                            kt_sb[m][hh * 64:(hh + 1) * 64,
                                     k * 128:(k + 1) * 128],
                            qt_sb[m][hh * 64:(hh + 1) * 64,
                                     q * 512:(q + 1) * 512],
                            start=True, stop=True,
                            tile_position=(hh * 64, 0))
                    pt = ptp.tile([128, 1024], BF16, tag="pt")
                    nc.scalar.activation(pt[:], ps[:], EXP)
                    nc.vector.tensor_tensor(
                        pt[:], pt[:],
                        mtd_sb[:, k * 2048 + q * 1024:k * 2048 + (q + 1) * 1024],
                        AluOpType.mult)
                    pts[k] = pt
                return pts

            def emit_attnv(m, q, pts):
                """attn@V' for group (m,q) + normalize into at_sb (bf16)."""
                pos = []
                for hh in range(2):
                    h = m * 2 + hh
                    po = psV.tile([DH + 1, 512], F32, tag="pv",
                                  name=f"po{hh}")
                    for k in range(KT):
                        nc.tensor.matmul(
                            po[:],
                            vp_sb[k][:, h * (DH + 1):(h + 1) * (DH + 1)],
                            pts[k][:, hh * 512:(hh + 1) * 512],
                            start=(k == 0), stop=(k == KT - 1))
                    pos.append(po)
                for hh in range(2):
                    # reciprocal_approx mis-addresses partition-offset PSUM
                    # inputs; stage the z row at partition 0 in SBUF first.
                    zrow = smallp.tile([1, 512], F32, tag="zrow",
                                       name=f"zr{hh}")
                    nc.vector.tensor_copy(zrow[:], pos[hh][DH:DH + 1, :])
                    rec = smallp.tile([1, 512], F32, tag="rec",
                                      name=f"rc{hh}")
                    nc.vector.reciprocal_approx_fast(rec[:], zrow[:])
                    bc = smallp.tile([64, 512], F32, tag="bc",
                                     name=f"bc{hh}")
                    nc.gpsimd.partition_broadcast(bc[:], rec[:])
                    nc.vector.tensor_tensor(
                        at_sb[m][hh * 64:(hh + 1) * 64,
                                 q * 512:(q + 1) * 512],
                        pos[hh][0:DH, :], bc[:], AluOpType.mult)

            def emit_outproj(dt, q):
                """out^T tile: psum[d_out 128, s 512] -> SBUF -> DMA out."""
                pc = psS.tile([128, 1024], F32, tag="ps", name=f"pc{dt}")
                pcs = pc[:, 0:512]
                for mc in range(4):
                    nc.tensor.matmul(
                        pcs,
                        wot_sb[:, mc * D + dt * 128:mc * D + (dt + 1) * 128],
                        at_sb[mc][:, q * 512:(q + 1) * 512],
                        start=(mc == 0), stop=(mc == 3))
                ot = smallp.tile([128, 512], F32, tag="ot", name=f"ot{dt}")
                nc.scalar.copy(ot[:], pcs)
                ring = nc.sync if (dt + q) % 2 == 0 else nc.scalar
                ring.dma_start(
                    d_out[dt * 128:(dt + 1) * 128, q * 512:(q + 1) * 512],
                    ot[:])

            # ---- static schedule ----
            # Prologue: heads m0, first scores early so ACT starts streaming,
            # then V / remaining projections fill PE while ACT works.
            emit_qk(0)
            sc = emit_scores(0, 0)
            for sb in range(8):
                emit_v(sb)
            emit_qk(1)
            prev = ((0, 0), sc)

            # Steady state: groups q-major; AV(g-1) + filler behind S(g).
            order = [(1, 0), (2, 0), (3, 0), (0, 1), (1, 1), (2, 1), (3, 1)]
            fillers = {(1, 0): lambda: emit_qk(2), (2, 0): lambda: emit_qk(3)}
            # C(q0) tiles interleave into the q1 groups: at iteration g the
            # emit_attnv(prev) call has just retired AV(3,0), so all at[*][:,
            # 0:512] slices are written before any C(*,0) read.
            outq0 = [(0, 1), (1, 1), (2, 1), (3, 1)]
            for g in order:
                sc = emit_scores(*g)
                emit_attnv(prev[0][0], prev[0][1], prev[1])
                if g in fillers:
                    fillers[g]()
                if g in outq0:
                    i = outq0.index(g)
                    emit_outproj(2 * i, 0)
                    emit_outproj(2 * i + 1, 0)
                prev = (g, sc)
            emit_attnv(prev[0][0], prev[0][1], prev[1])
            for dt in range(8):
                emit_outproj(dt, 1)

    nc.compile()
    return nc


_prog = None


def _get_prog():
    global _prog
    if _prog is None:
        _prog = build_program()
    return _prog


def _host_prep(x, prior_mask, prior_indices, prior_index_mask, u_prev,
               Wq, bq, Wk, bk, Wv, bv, Wo, bo):
    f32 = np.float32
    bf16 = ml_dtypes.bfloat16
    x = np.asarray(x, f32)
    pm = np.asarray(prior_mask, bool)
    idx = np.asarray(prior_indices)
    pim = np.asarray(prior_index_mask, bool)
    u = np.asarray(u_prev, f32).reshape(B)
    Wq, Wk, Wv, Wo = (np.asarray(w, f32) for w in (Wq, Wk, Wv, Wo))
    bq, bk, bv, bo = (np.asarray(v, f32) for v in (bq, bk, bv, bo))

    scale = f32(1.0 / np.sqrt(DH))
    lam = (LAMBDA_MAX * np.exp(-ALPHA * u.astype(np.float64))).astype(f32)
    use_sparse = lam >= SPARSE_THRESHOLD

    # Multiplicative prior M^T [k, q], duplicated per q-half for the
    # [hh0|hh1] P-tile layout: mtd[:, q*1024:(q+1)*1024] = [Mt_q | Mt_q].
    mtd_sparse = None
    if use_sparse.any():
        cnt = np.zeros((S, S + 1), f32)
        np.add.at(cnt, (np.arange(S)[:, None],
                        np.where(pim, idx, S).astype(np.int64)), 1.0)
        mt = np.ascontiguousarray(cnt[:, :S].T)
        mtd_sparse = np.concatenate(
            [mt[:, :512], mt[:, :512], mt[:, 512:], mt[:, 512:]],
            axis=1).astype(bf16)

    mtds = []
    for b in range(B):
        if use_sparse[b]:
            mtds.append(mtd_sparse)
        else:
            mt = np.where(pm, f32(1.0), np.exp(-lam[b], dtype=f32)).T
            mtds.append(np.concatenate(
                [mt[:, :512], mt[:, :512], mt[:, 512:], mt[:, 512:]],
                axis=1).astype(bf16))

    in_maps = []
    for c in range(N_CORES):
        b = c // 2
        hg = c % 2
        hsl = slice(hg * 512, (hg + 1) * 512)
        in_maps.append({
            "xt": np.ascontiguousarray(x[b].T).astype(bf16),
            "wqt": np.ascontiguousarray((Wq[hsl] * scale).T).astype(bf16),
            "wkt": np.ascontiguousarray(Wk[hsl].T).astype(bf16),
            "wvt": np.ascontiguousarray(Wv[hsl].T).astype(bf16),
            "wot": np.ascontiguousarray(Wo[:, hsl].T).astype(bf16),
            "mtd": mtds[b],
            "bq": np.ascontiguousarray((bq[hsl] * scale).reshape(4, 128).T),
            "bk": np.ascontiguousarray(bk[hsl].reshape(4, 128).T),
        })
    return in_maps


def kernel(**inputs):
    in_maps = _host_prep(**inputs)
    nc = _get_prog()
    res = run_bass_kernel_spmd(nc, in_maps, core_ids=list(range(N_CORES)))
    # Constant bias terms pass through the attention average unchanged:
    # out += bo + bv @ Wo^T  (z-normalized ones-column makes bv exact).
    bv = np.asarray(inputs["bv"], np.float32)
    bo = np.asarray(inputs["bo"], np.float32)
    Wo = np.asarray(inputs["Wo"], np.float32)
    const_row = bo + bv @ Wo.T
    out = np.empty((B, S, D), np.float32)
    for b in range(B):
        pt = res.results[2 * b]["out"] + res.results[2 * b + 1]["out"]
        out[b] = pt.T + const_row
    return out


# revision 14
# speedup vs baseline: 1.2829x; 1.0392x over previous
"""Trainium2 Bass kernel for ClippingAttentionEngine.

Sharding: core c -> (batch b = c//2, head-group hg = c%2, 8 heads each).
Each core computes Q/K/V projections for its 8 heads, attention, and the
partial transposed output projection over its head slice; host sums the two
per-batch partials, transposes, and adds the constant bias terms
(bo + bv @ Wo^T -- the V bias passes through softmax averaging unchanged).

The per-sample sparse/dense branch is folded into a single dense-shaped
program via a MULTIPLICATIVE prior M (host-built, bf16):
  dense batch:  M[q,k] = pm[q,k] ? 1 : exp(-lambda)
  sparse batch: M[q,k] = multiplicity of key k in prior_indices[q] (masked
                slots excluded), so P = exp(s) * M reproduces the gathered
                sparse softmax exactly (duplicates included, 0 = exact mask).

All matmuls are bf16 (fp32 PSUM accumulate). Engine assignment:
  PE   : projections, scores (row-tiled hh pairs run concurrently),
         attn@V' (ones-column gives the softmax denominator), out-proj
  ACT  : exp (exp table stays loaded) + stage-C PSUM->SBUF copies
  DVE  : Q/K PSUM->SBUF copies w/ bias, P = exp(s)*M multiply (2x bf16),
         z-row staging + reciprocal
  Pool : softmax normalize multiplies, V copies, z broadcast, memsets
Inputs arrive as a few large merged DMAs split across both HWDGE rings
(sync + scalar) to cut ring serialization; outputs alternate rings.
"""

import sys

sys.path.insert(0, "/opt/trn_rl_repo")

import ml_dtypes
import numpy as np

import concourse.bass as bass
import concourse.tile as tile
from concourse import bacc, mybir
from concourse.alu_op_type import AluOpType
from concourse.bass_utils import run_bass_kernel_spmd

B, S, D, H = 4, 1024, 1024, 16
DH = D // H          # 64
HPC = 8              # heads per core
N_CORES = 8
KT = S // 128        # 8 k tiles
DCH = D // 128       # 8 contraction chunks
LAMBDA_MAX, ALPHA, SPARSE_THRESHOLD = 10.0, 5.0, 1.0

F32 = mybir.dt.float32
BF16 = mybir.dt.bfloat16
EXP = mybir.ActivationFunctionType.Exp


def build_program():
    nc = bacc.Bacc("TRN2", target_bir_lowering=False, debug=False,
                   num_devices=N_CORES)

    d_xt = nc.dram_tensor("xt", [D, S], BF16, kind="ExternalInput").ap()
    d_wqt = nc.dram_tensor("wqt", [D, 512], BF16, kind="ExternalInput").ap()
    d_wkt = nc.dram_tensor("wkt", [D, 512], BF16, kind="ExternalInput").ap()
    d_wvt = nc.dram_tensor("wvt", [D, 512], BF16, kind="ExternalInput").ap()
    d_wot = nc.dram_tensor("wot", [512, D], BF16, kind="ExternalInput").ap()
    d_mtd = nc.dram_tensor("mtd", [S, 1024], BF16, kind="ExternalInput").ap()
    d_bq = nc.dram_tensor("bq", [128, 4], F32, kind="ExternalInput").ap()
    d_bk = nc.dram_tensor("bk", [128, 4], F32, kind="ExternalInput").ap()
    d_out = nc.dram_tensor("out", [D, S], F32, kind="ExternalOutput").ap()

    with tile.TileContext(nc) as tc:
        with (
            tc.tile_pool(name="const", bufs=1) as constp,
            tc.tile_pool(name="main", bufs=1) as mainp,
            tc.tile_pool(name="inp", bufs=1) as inp,
            tc.tile_pool(name="ptp", bufs=16) as ptp,
            tc.tile_pool(name="smallp", bufs=4) as smallp,
            tc.tile_pool(name="psS", bufs=3, space="PSUM") as psS,
            tc.tile_pool(name="psV", bufs=2, space="PSUM") as psV,
        ):
            bq_sb = constp.tile([128, 4], F32, tag="bq")
            nc.scalar.dma_start(bq_sb[:], d_bq[:])
            bk_sb = constp.tile([128, 4], F32, tag="bk")
            nc.scalar.dma_start(bk_sb[:], d_bk[:])

            # Persistent arrays.
            qt_sb = [mainp.tile([128, S], BF16, tag=f"qt{m}", name=f"qt{m}")
                     for m in range(4)]
            kt_sb = [mainp.tile([128, S], BF16, tag=f"kt{m}", name=f"kt{m}")
                     for m in range(4)]
            vp_sb = [mainp.tile([128, HPC * (DH + 1)], BF16, tag=f"vp{sb}",
                                name=f"vp{sb}") for sb in range(8)]
            at_sb = [mainp.tile([128, S], BF16, tag=f"at{m}", name=f"at{m}")
                     for m in range(4)]
            wot_sb = mainp.tile([128, 4 * D], BF16, tag="wot", name="wot")
            mtd_sb = mainp.tile([128, KT * 1024], BF16, tag="mtd", name="mtd")

            # Stage-A inputs, merged for large DMAs.
            xt_sb = inp.tile([128, DCH * S], BF16, tag="xt", name="xt")
            w_sb = {nm: inp.tile([128, DCH * 512], BF16, tag=f"w{nm}",
                                 name=f"w{nm}") for nm in ("q", "k", "v")}

            def chunked(dst, src, n, csz, ring, pieces):
                """DMA dram [n*128, csz] -> sbuf [128, n*csz] in `pieces`."""
                d3 = dst.rearrange("p (c s) -> p c s", s=csz)
                s3 = src.rearrange("(c p) s -> p c s", p=128)
                step = n // pieces
                for i in range(pieces):
                    sl = slice(i * step, (i + 1) * step)
                    ring.dma_start(d3[:, sl], s3[:, sl])

            # The 16 SDMA engines round-robin both rings' packets, so issue
            # order ~= completion order across BOTH rings. Interleave pieces
            # by first use: qk(0) streams xt/wq/wk chunk-by-chunk, then
            # S(0,0)'s multiplies need mtd, then V needs wv, stage C wot.
            def piece(dst, src, n, csz, ring, lo, hi):
                d3 = dst.rearrange("p (c s) -> p c s", s=csz)
                s3 = src.rearrange("(c p) s -> p c s", p=128)
                ring.dma_start(d3[:, lo:hi], s3[:, lo:hi])

            piece(xt_sb, d_xt, DCH, S, nc.sync, 0, 2)
            piece(w_sb["q"], d_wqt, DCH, 512, nc.scalar, 0, 4)
            piece(xt_sb, d_xt, DCH, S, nc.sync, 2, 4)
            piece(w_sb["q"], d_wqt, DCH, 512, nc.scalar, 4, 8)
            piece(xt_sb, d_xt, DCH, S, nc.sync, 4, 6)
            piece(w_sb["k"], d_wkt, DCH, 512, nc.scalar, 0, 4)
            piece(xt_sb, d_xt, DCH, S, nc.sync, 6, 8)
            piece(w_sb["k"], d_wkt, DCH, 512, nc.scalar, 4, 8)
            for j in range(4):
                piece(mtd_sb, d_mtd, KT, 1024, nc.sync, 2 * j, 2 * j + 2)
            piece(w_sb["v"], d_wvt, DCH, 512, nc.scalar, 0, 4)
            piece(w_sb["v"], d_wvt, DCH, 512, nc.scalar, 4, 8)
            piece(wot_sb, d_wot, 4, D, nc.sync, 0, 4)

            # Ones columns of vp (softmax denominator rows): set once.
            for sb in range(8):
                vp3 = vp_sb[sb].rearrange("p (h d) -> p h d", d=DH + 1)
                nc.gpsimd.memset(vp3[:, :, DH:DH + 1], 1.0)

            # ---- emission helpers ----
            def emit_qk(m):
                """Q^T,K^T head-pair m: psum[d'128, s1024]; DVE copy+bias."""
                for nm, dst, bias in (("q", qt_sb, bq_sb), ("k", kt_sb, bk_sb)):
                    pp = psS.tile([128, 1024], F32, tag="ps", name=f"pp{nm}")
                    for st in range(2):
                        for c in range(DCH):
                            nc.tensor.matmul(
                                pp[:, st * 512:(st + 1) * 512],
                                w_sb[nm][:, c * 512 + m * 128:
                                         c * 512 + (m + 1) * 128],
                                xt_sb[:, c * S + st * 512:
                                      c * S + (st + 1) * 512],
                                start=(c == 0), stop=(c == DCH - 1))
                    nc.vector.tensor_scalar_add(dst[m][:], pp[:],
                                                bias[:, m:m + 1])

            def emit_v(sb):
                """V block sb: psum[s128, dh512] -> vp (strided bf16 copy)."""
                pv = psV.tile([128, 512], F32, tag="pv", name="pv")
                for c in range(DCH):
                    nc.tensor.matmul(
                        pv[:],
                        xt_sb[:, c * S + sb * 128:c * S + (sb + 1) * 128],
                        w_sb["v"][:, c * 512:(c + 1) * 512],
                        start=(c == 0), stop=(c == DCH - 1))
                vp3 = vp_sb[sb].rearrange("p (h d) -> p h d", d=DH + 1)
                nc.vector.tensor_copy(
                    vp3[:, :, 0:DH],
                    pv[:].rearrange("p (h d) -> p h d", d=DH))

            def emit_scores(m, q):
                """Scores k-tiles for group (m,q): P^T = exp(K^T.T@Q^T) * M."""
                pts = {}
                for k in range(KT):
                    ps = psS.tile([128, 1024], F32, tag="ps", name="ps")
                    for hh in range(2):
                        nc.tensor.matmul(
                            ps[:, hh * 512:(hh + 1) * 512],
                            kt_sb[m][hh * 64:(hh + 1) * 64,
                                     k * 128:(k + 1) * 128],
                            qt_sb[m][hh * 64:(hh + 1) * 64,
                                     q * 512:(q + 1) * 512],
                            start=True, stop=True,
                            tile_position=(hh * 64, 0))
                    pt = ptp.tile([128, 1024], BF16, tag="pt")
                    nc.scalar.activation(pt[:], ps[:], EXP)
                    # M slice broadcast across the two hh halves (stride-0).
                    msl = mtd_sb[:, k * 1024 + q * 512:k * 1024 + (q + 1) * 512]
                    nc.vector.tensor_tensor(
                        pt[:].rearrange("p (two s) -> p two s", two=2),
                        pt[:].rearrange("p (two s) -> p two s", two=2),
                        msl.rearrange("p (one s) -> p one s",
                                      one=1).broadcast_to((128, 2, 512)),
                        AluOpType.mult)
                    pts[k] = pt
                return pts

            def emit_attnv(m, q, pts):
                """attn@V' for group (m,q) + normalize into at_sb (bf16)."""
                pos = []
                for hh in range(2):
                    h = m * 2 + hh
                    po = psV.tile([DH + 1, 512], F32, tag="pv",
                                  name=f"po{hh}")
                    for k in range(KT):
                        nc.tensor.matmul(
                            po[:],
                            vp_sb[k][:, h * (DH + 1):(h + 1) * (DH + 1)],
                            pts[k][:, hh * 512:(hh + 1) * 512],
                            start=(k == 0), stop=(k == KT - 1))
                    pos.append(po)
                for hh in range(2):
                    # reciprocal_approx mis-addresses partition-offset PSUM
                    # inputs; stage the z row at partition 0 in SBUF first.
                    zrow = smallp.tile([1, 512], F32, tag="zrow",
                                       name=f"zr{hh}")
                    nc.vector.tensor_copy(zrow[:], pos[hh][DH:DH + 1, :])
                    rec = smallp.tile([1, 512], F32, tag="rec",
                                      name=f"rc{hh}")
                    nc.vector.reciprocal_approx_fast(rec[:], zrow[:])
                    bc = smallp.tile([64, 512], F32, tag="bc",
                                     name=f"bc{hh}")
                    nc.gpsimd.partition_broadcast(bc[:], rec[:])
                    nc.vector.tensor_tensor(
                        at_sb[m][hh * 64:(hh + 1) * 64,
                                 q * 512:(q + 1) * 512],
                        pos[hh][0:DH, :], bc[:], AluOpType.mult)

            def emit_outproj(dt, q):
                """out^T tile: psum[d_out 128, s 512] -> SBUF -> DMA out."""
                pc = psS.tile([128, 1024], F32, tag="ps", name=f"pc{dt}")
                pcs = pc[:, 0:512]
                for mc in range(4):
                    nc.tensor.matmul(
                        pcs,
                        wot_sb[:, mc * D + dt * 128:mc * D + (dt + 1) * 128],
                        at_sb[mc][:, q * 512:(q + 1) * 512],
                        start=(mc == 0), stop=(mc == 3))
                ot = smallp.tile([128, 512], F32, tag="ot", name=f"ot{dt}")
                nc.scalar.copy(ot[:], pcs)
                ring = nc.sync if (dt + q) % 2 == 0 else nc.scalar
                ring.dma_start(
                    d_out[dt * 128:(dt + 1) * 128, q * 512:(q + 1) * 512],
                    ot[:])

            # ---- static schedule ----
            # Prologue: heads m0, first scores early so ACT starts streaming,
            # then V / remaining projections fill PE while ACT works.
            emit_qk(0)
            sc = emit_scores(0, 0)
            for sb in range(8):
                emit_v(sb)
            emit_qk(1)
            prev = ((0, 0), sc)

            # Steady state: groups q-major; AV(g-1) + filler behind S(g).
            order = [(1, 0), (2, 0), (3, 0), (0, 1), (1, 1), (2, 1), (3, 1)]
            fillers = {(1, 0): lambda: emit_qk(2), (2, 0): lambda: emit_qk(3)}
            # C(q0) tiles interleave into the q1 groups: at iteration g the
            # emit_attnv(prev) call has just retired AV(3,0), so all at[*][:,
            # 0:512] slices are written before any C(*,0) read.
            outq0 = [(0, 1), (1, 1), (2, 1), (3, 1)]
            for g in order:
                sc = emit_scores(*g)
                emit_attnv(prev[0][0], prev[0][1], prev[1])
                if g in fillers:
                    fillers[g]()
                if g in outq0:
                    i = outq0.index(g)
                    emit_outproj(2 * i, 0)
                    emit_outproj(2 * i + 1, 0)
                prev = (g, sc)
            emit_attnv(prev[0][0], prev[0][1], prev[1])
            for dt in range(8):
                emit_outproj(dt, 1)

    nc.compile()
    return nc


_prog = None


def _get_prog():
    global _prog
    if _prog is None:
        _prog = build_program()
    return _prog


def _host_prep(x, prior_mask, prior_indices, prior_index_mask, u_prev,
               Wq, bq, Wk, bk, Wv, bv, Wo, bo):
    f32 = np.float32
    bf16 = ml_dtypes.bfloat16
    x = np.asarray(x, f32)
    pm = np.asarray(prior_mask, bool)
    idx = np.asarray(prior_indices)
    pim = np.asarray(prior_index_mask, bool)
    u = np.asarray(u_prev, f32).reshape(B)
    Wq, Wk, Wv, Wo = (np.asarray(w, f32) for w in (Wq, Wk, Wv, Wo))
    bq, bk, bv, bo = (np.asarray(v, f32) for v in (bq, bk, bv, bo))

    scale = f32(1.0 / np.sqrt(DH))
    lam = (LAMBDA_MAX * np.exp(-ALPHA * u.astype(np.float64))).astype(f32)
    use_sparse = lam >= SPARSE_THRESHOLD

    # Multiplicative prior M^T [k, q] (device broadcasts per q-half).
    mtd_sparse = None
    if use_sparse.any():
        cnt = np.zeros((S, S + 1), f32)
        np.add.at(cnt, (np.arange(S)[:, None],
                        np.where(pim, idx, S).astype(np.int64)), 1.0)
        mtd_sparse = np.ascontiguousarray(cnt[:, :S].T).astype(bf16)

    mtds = []
    for b in range(B):
        if use_sparse[b]:
            mtds.append(mtd_sparse)
        else:
            mt = np.where(pm, f32(1.0), np.exp(-lam[b], dtype=f32)).T
            mtds.append(np.ascontiguousarray(mt).astype(bf16))

    in_maps = []
    for c in range(N_CORES):
        b = c // 2
        hg = c % 2
        hsl = slice(hg * 512, (hg + 1) * 512)
        in_maps.append({
            "xt": np.ascontiguousarray(x[b].T).astype(bf16),
            "wqt": np.ascontiguousarray((Wq[hsl] * scale).T).astype(bf16),
            "wkt": np.ascontiguousarray(Wk[hsl].T).astype(bf16),
            "wvt": np.ascontiguousarray(Wv[hsl].T).astype(bf16),
            "wot": np.ascontiguousarray(Wo[:, hsl].T).astype(bf16),
            "mtd": mtds[b],
            "bq": np.ascontiguousarray((bq[hsl] * scale).reshape(4, 128).T),
            "bk": np.ascontiguousarray(bk[hsl].reshape(4, 128).T),
        })
    return in_maps


def kernel(**inputs):
    in_maps = _host_prep(**inputs)
    nc = _get_prog()
    res = run_bass_kernel_spmd(nc, in_maps, core_ids=list(range(N_CORES)))
    # Constant bias terms pass through the attention average unchanged:
    # out += bo + bv @ Wo^T  (z-normalized ones-column makes bv exact).
    bv = np.asarray(inputs["bv"], np.float32)
    bo = np.asarray(inputs["bo"], np.float32)
    Wo = np.asarray(inputs["Wo"], np.float32)
    const_row = bo + bv @ Wo.T
    out = np.empty((B, S, D), np.float32)
    for b in range(B):
        pt = res.results[2 * b]["out"] + res.results[2 * b + 1]["out"]
        out[b] = pt.T + const_row
    return out


# revision 20
# speedup vs baseline: 1.2853x; 1.0019x over previous
"""Trainium2 Bass kernel for ClippingAttentionEngine.

Sharding: core c -> (batch b = c//2, head-group hg = c%2, 8 heads each).
Each core computes Q/K/V projections for its 8 heads, attention, and the
partial transposed output projection over its head slice; host sums the two
per-batch partials, transposes, and adds the constant bias terms
(bo + bv @ Wo^T -- the V bias passes through softmax averaging unchanged).

The per-sample sparse/dense branch is folded into a single dense-shaped
program via a MULTIPLICATIVE prior M (host-built, bf16):
  dense batch:  M[q,k] = pm[q,k] ? 1 : exp(-lambda)
  sparse batch: M[q,k] = multiplicity of key k in prior_indices[q] (masked
                slots excluded), so P = exp(s) * M reproduces the gathered
                sparse softmax exactly (duplicates included, 0 = exact mask).

All matmuls are bf16 (fp32 PSUM accumulate). Engine assignment:
  PE   : projections, scores (row-tiled hh pairs run concurrently),
         attn@V' (ones-column gives the softmax denominator), out-proj
  ACT  : exp (exp table stays loaded) + stage-C PSUM->SBUF copies
  DVE  : Q/K PSUM->SBUF copies w/ bias, P = exp(s)*M multiply (2x bf16),
         z-row staging + reciprocal
  Pool : softmax normalize multiplies, V copies, z broadcast, memsets
Inputs arrive as a few large merged DMAs split across both HWDGE rings
(sync + scalar) to cut ring serialization; outputs alternate rings.
"""

import sys

sys.path.insert(0, "/opt/trn_rl_repo")

import ml_dtypes
import numpy as np

import concourse.bass as bass
import concourse.tile as tile
from concourse import bacc, mybir
from concourse.alu_op_type import AluOpType
from concourse.bass_utils import run_bass_kernel_spmd

B, S, D, H = 4, 1024, 1024, 16
DH = D // H          # 64
HPC = 8              # heads per core
N_CORES = 8
KT = S // 128        # 8 k tiles
DCH = D // 128       # 8 contraction chunks
LAMBDA_MAX, ALPHA, SPARSE_THRESHOLD = 10.0, 5.0, 1.0

F32 = mybir.dt.float32
BF16 = mybir.dt.bfloat16
EXP = mybir.ActivationFunctionType.Exp


def build_program():
    nc = bacc.Bacc("TRN2", target_bir_lowering=False, debug=False,
                   num_devices=N_CORES)

    d_xt = nc.dram_tensor("xt", [D, S], BF16, kind="ExternalInput").ap()
    d_wqt = nc.dram_tensor("wqt", [D, 512], BF16, kind="ExternalInput").ap()
    d_wkt = nc.dram_tensor("wkt", [D, 512], BF16, kind="ExternalInput").ap()
    d_wvt = nc.dram_tensor("wvt", [D, 512], BF16, kind="ExternalInput").ap()
    d_wot = nc.dram_tensor("wot", [512, D], BF16, kind="ExternalInput").ap()
    d_mtd = nc.dram_tensor("mtd", [S, 1024], BF16, kind="ExternalInput").ap()
    d_bq = nc.dram_tensor("bq", [128, 4], F32, kind="ExternalInput").ap()
    d_bk = nc.dram_tensor("bk", [128, 4], F32, kind="ExternalInput").ap()
    d_out = nc.dram_tensor("out", [D, S], F32, kind="ExternalOutput").ap()

    with tile.TileContext(nc) as tc:
        with (
            tc.tile_pool(name="const", bufs=1) as constp,
            tc.tile_pool(name="main", bufs=1) as mainp,
            tc.tile_pool(name="inp", bufs=1) as inp,
            tc.tile_pool(name="ptp", bufs=16) as ptp,
            tc.tile_pool(name="smallp", bufs=4) as smallp,
            tc.tile_pool(name="psS", bufs=3, space="PSUM") as psS,
            tc.tile_pool(name="psV", bufs=2, space="PSUM") as psV,
        ):
            bq_sb = constp.tile([128, 4], F32, tag="bq")
            nc.scalar.dma_start(bq_sb[:], d_bq[:])
            bk_sb = constp.tile([128, 4], F32, tag="bk")
            nc.scalar.dma_start(bk_sb[:], d_bk[:])

            # Persistent arrays.
            qt_sb = [mainp.tile([128, S], BF16, tag=f"qt{m}", name=f"qt{m}")
                     for m in range(4)]
            kt_sb = [mainp.tile([128, S], BF16, tag=f"kt{m}", name=f"kt{m}")
                     for m in range(4)]
            vp_sb = [mainp.tile([128, HPC * (DH + 1)], BF16, tag=f"vp{sb}",
                                name=f"vp{sb}") for sb in range(8)]
            at_sb = [mainp.tile([128, S], BF16, tag=f"at{m}", name=f"at{m}")
                     for m in range(4)]
            wot_sb = mainp.tile([128, 4 * D], BF16, tag="wot", name="wot")

            # Stage-A inputs: one SBUF tile per DMA piece so dependency
            # tracking stays piece-granular (a merged tile would gate the
            # first matmul on the LAST piece's DMA).
            xt_t = [inp.tile([128, 2 * S], BF16, tag=f"xt{j}", name=f"xt{j}")
                    for j in range(4)]
            w_t = {nm: [inp.tile([128, 4 * 512], BF16, tag=f"w{nm}{j}",
                                 name=f"w{nm}{j}") for j in range(2)]
                   for nm in ("q", "k", "v")}
            mtd_t = [inp.tile([128, 2 * 1024], BF16, tag=f"mtd{j}",
                              name=f"mtd{j}") for j in range(4)]

            def xt_ap(c, lo, hi):
                return xt_t[c // 2][:, (c % 2) * S + lo:(c % 2) * S + hi]

            def w_ap(nm, c, lo, hi):
                return w_t[nm][c // 4][:, (c % 4) * 512 + lo:
                                       (c % 4) * 512 + hi]

            def mtd_ap(k, lo, hi):
                return mtd_t[k // 2][:, (k % 2) * 1024 + lo:
                                     (k % 2) * 1024 + hi]

            def chunked(dst, src, n, csz, ring, pieces):
                """DMA dram [n*128, csz] -> sbuf [128, n*csz] in `pieces`."""
                d3 = dst.rearrange("p (c s) -> p c s", s=csz)
                s3 = src.rearrange("(c p) s -> p c s", p=128)
                step = n // pieces
                for i in range(pieces):
                    sl = slice(i * step, (i + 1) * step)
                    ring.dma_start(d3[:, sl], s3[:, sl])

            # The 16 SDMA engines round-robin both rings' packets; each ring
            # is FIFO. Interleave pieces by first use: qk(0) streams xt/wq/wk
            # chunk-by-chunk, then S(0,0)'s multiplies need mtd, then V
            # needs wv, stage C wot.
            def piece(dst, src, csz, ring, lo, hi):
                d3 = dst.rearrange("p (c s) -> p c s", s=csz)
                s3 = src.rearrange("(c p) s -> p c s", p=128)
                ring.dma_start(d3[:], s3[:, lo:hi])

            piece(xt_t[0], d_xt, S, nc.sync, 0, 2)
            piece(w_t["q"][0], d_wqt, 512, nc.scalar, 0, 4)
            piece(xt_t[1], d_xt, S, nc.sync, 2, 4)
            piece(w_t["q"][1], d_wqt, 512, nc.scalar, 4, 8)
            piece(xt_t[2], d_xt, S, nc.sync, 4, 6)
            piece(w_t["k"][0], d_wkt, 512, nc.scalar, 0, 4)
            piece(xt_t[3], d_xt, S, nc.sync, 6, 8)
            piece(w_t["k"][1], d_wkt, 512, nc.scalar, 4, 8)
            for j in range(4):
                piece(mtd_t[j], d_mtd, 1024, nc.sync, 2 * j, 2 * j + 2)
            piece(w_t["v"][0], d_wvt, 512, nc.scalar, 0, 4)
            piece(w_t["v"][1], d_wvt, 512, nc.scalar, 4, 8)
            piece(wot_sb, d_wot, D, nc.sync, 0, 4)

            # Ones columns of vp (softmax denominator rows): set once.
            for sb in range(8):
                vp3 = vp_sb[sb].rearrange("p (h d) -> p h d", d=DH + 1)
                nc.gpsimd.memset(vp3[:, :, DH:DH + 1], 1.0)

            # ---- emission helpers ----
            def emit_qk(m):
                """Q^T,K^T head-pair m: psum[d'128, s1024]; DVE copy+bias."""
                for nm, dst, bias in (("q", qt_sb, bq_sb), ("k", kt_sb, bk_sb)):
                    pp = psS.tile([128, 1024], F32, tag="ps", name=f"pp{nm}")
                    for st in range(2):
                        for c in range(DCH):
                            nc.tensor.matmul(
                                pp[:, st * 512:(st + 1) * 512],
                                w_ap(nm, c, m * 128, (m + 1) * 128),
                                xt_ap(c, st * 512, (st + 1) * 512),
                                start=(c == 0), stop=(c == DCH - 1))
                    nc.vector.tensor_scalar_add(dst[m][:], pp[:],
                                                bias[:, m:m + 1])

            def emit_v(sb):
                """V block sb: psum[s128, dh512] -> vp (strided bf16 copy)."""
                pv = psV.tile([128, 512], F32, tag="pv", name="pv")
                for c in range(DCH):
                    nc.tensor.matmul(
                        pv[:],
                        xt_ap(c, sb * 128, (sb + 1) * 128),
                        w_ap("v", c, 0, 512),
                        start=(c == 0), stop=(c == DCH - 1))
                vp3 = vp_sb[sb].rearrange("p (h d) -> p h d", d=DH + 1)
                nc.vector.tensor_copy(
                    vp3[:, :, 0:DH],
                    pv[:].rearrange("p (h d) -> p h d", d=DH))

            def emit_scores(m, q):
                """Scores k-tiles for group (m,q): P^T = exp(K^T.T@Q^T) * M."""
                pts = {}
                for k in range(KT):
                    ps = psS.tile([128, 1024], F32, tag="ps", name="ps")
                    for hh in range(2):
                        nc.tensor.matmul(
                            ps[:, hh * 512:(hh + 1) * 512],
                            kt_sb[m][hh * 64:(hh + 1) * 64,
                                     k * 128:(k + 1) * 128],
                            qt_sb[m][hh * 64:(hh + 1) * 64,
                                     q * 512:(q + 1) * 512],
                            start=True, stop=True,
                            tile_position=(hh * 64, 0))
                    pt = ptp.tile([128, 1024], BF16, tag="pt")
                    nc.scalar.activation(pt[:], ps[:], EXP)
                    # M slice broadcast across the two hh halves (stride-0).
                    msl = mtd_ap(k, q * 512, (q + 1) * 512)
                    nc.vector.tensor_tensor(
                        pt[:].rearrange("p (two s) -> p two s", two=2),
                        pt[:].rearrange("p (two s) -> p two s", two=2),
                        msl.rearrange("p (one s) -> p one s",
                                      one=1).broadcast_to((128, 2, 512)),
                        AluOpType.mult)
                    pts[k] = pt
                return pts

            def emit_attnv(m, q, pts):
                """attn@V' for group (m,q) + normalize into at_sb (bf16)."""
                pos = []
                for hh in range(2):
                    h = m * 2 + hh
                    po = psV.tile([DH + 1, 512], F32, tag="pv",
                                  name=f"po{hh}")
                    for k in range(KT):
                        nc.tensor.matmul(
                            po[:],
                            vp_sb[k][:, h * (DH + 1):(h + 1) * (DH + 1)],
                            pts[k][:, hh * 512:(hh + 1) * 512],
                            start=(k == 0), stop=(k == KT - 1))
                    pos.append(po)
                for hh in range(2):
                    # reciprocal_approx mis-addresses partition-offset PSUM
                    # inputs; stage the z row at partition 0 in SBUF first.
                    zrow = smallp.tile([1, 512], F32, tag="zrow",
                                       name=f"zr{hh}")
                    nc.vector.tensor_copy(zrow[:], pos[hh][DH:DH + 1, :])
                    rec = smallp.tile([1, 512], F32, tag="rec",
                                      name=f"rc{hh}")
                    nc.vector.reciprocal_approx_fast(rec[:], zrow[:])
                    bc = smallp.tile([64, 512], F32, tag="bc",
                                     name=f"bc{hh}")
                    nc.gpsimd.partition_broadcast(bc[:], rec[:])
                    nc.vector.tensor_tensor(
                        at_sb[m][hh * 64:(hh + 1) * 64,
                                 q * 512:(q + 1) * 512],
                        pos[hh][0:DH, :], bc[:], AluOpType.mult)

            def emit_outproj(dt, q):
                """out^T tile: psum[d_out 128, s 512] -> SBUF -> DMA out."""
                pc = psS.tile([128, 1024], F32, tag="ps", name=f"pc{dt}")
                pcs = pc[:, 0:512]
                for mc in range(4):
                    nc.tensor.matmul(
                        pcs,
                        wot_sb[:, mc * D + dt * 128:mc * D + (dt + 1) * 128],
                        at_sb[mc][:, q * 512:(q + 1) * 512],
                        start=(mc == 0), stop=(mc == 3))
                ot = smallp.tile([128, 512], F32, tag="ot", name=f"ot{dt}")
                nc.scalar.copy(ot[:], pcs)
                ring = nc.sync if (dt + q) % 2 == 0 else nc.scalar
                ring.dma_start(
                    d_out[dt * 128:(dt + 1) * 128, q * 512:(q + 1) * 512],
                    ot[:])

            # ---- static schedule ----
            # Prologue: heads m0, first scores early so ACT starts streaming,
            # then V / remaining projections fill PE while ACT works.
            emit_qk(0)
            sc = emit_scores(0, 0)
            for sb in range(8):
                emit_v(sb)
            emit_qk(1)
            prev = ((0, 0), sc)

            # Steady state: groups q-major; AV(g-1) + filler behind S(g).
            order = [(1, 0), (2, 0), (3, 0), (0, 1), (1, 1), (2, 1), (3, 1)]
            fillers = {(1, 0): lambda: emit_qk(2), (2, 0): lambda: emit_qk(3)}
            # C(q0) tiles interleave into the q1 groups: at iteration g the
            # emit_attnv(prev) call has just retired AV(3,0), so all at[*][:,
            # 0:512] slices are written before any C(*,0) read.
            outq0 = [(0, 1), (1, 1), (2, 1), (3, 1)]
            for g in order:
                sc = emit_scores(*g)
                emit_attnv(prev[0][0], prev[0][1], prev[1])
                if g in fillers:
                    fillers[g]()
                if g in outq0:
                    i = outq0.index(g)
                    emit_outproj(2 * i, 0)
                    emit_outproj(2 * i + 1, 0)
                prev = (g, sc)
            emit_attnv(prev[0][0], prev[0][1], prev[1])
            for dt in range(8):
                emit_outproj(dt, 1)

    nc.compile()
    return nc


_prog = None


def _get_prog():
    global _prog
    if _prog is None:
        _prog = build_program()
    return _prog


def _host_prep(x, prior_mask, prior_indices, prior_index_mask, u_prev,
               Wq, bq, Wk, bk, Wv, bv, Wo, bo):
    f32 = np.float32
    bf16 = ml_dtypes.bfloat16
    x = np.asarray(x, f32)
    pm = np.asarray(prior_mask, bool)
    idx = np.asarray(prior_indices)
    pim = np.asarray(prior_index_mask, bool)
    u = np.asarray(u_prev, f32).reshape(B)
    Wq, Wk, Wv, Wo = (np.asarray(w, f32) for w in (Wq, Wk, Wv, Wo))
    bq, bk, bv, bo = (np.asarray(v, f32) for v in (bq, bk, bv, bo))

    scale = f32(1.0 / np.sqrt(DH))
    lam = (LAMBDA_MAX * np.exp(-ALPHA * u.astype(np.float64))).astype(f32)
    use_sparse = lam >= SPARSE_THRESHOLD

    # Multiplicative prior M^T [k, q] (device broadcasts per q-half).
    mtd_sparse = None
    if use_sparse.any():
        cnt = np.zeros((S, S + 1), f32)
        np.add.at(cnt, (np.arange(S)[:, None],
                        np.where(pim, idx, S).astype(np.int64)), 1.0)
        mtd_sparse = np.ascontiguousarray(cnt[:, :S].T).astype(bf16)

    mtds = []
    for b in range(B):
        if use_sparse[b]:
            mtds.append(mtd_sparse)
        else:
            mt = np.where(pm, f32(1.0), np.exp(-lam[b], dtype=f32)).T
            mtds.append(np.ascontiguousarray(mt).astype(bf16))

    in_maps = []
    for c in range(N_CORES):
        b = c // 2
        hg = c % 2
        hsl = slice(hg * 512, (hg + 1) * 512)
        in_maps.append({
            "xt": np.ascontiguousarray(x[b].T).astype(bf16),
            "wqt": np.ascontiguousarray((Wq[hsl] * scale).T).astype(bf16),
            "wkt": np.ascontiguousarray(Wk[hsl].T).astype(bf16),
            "wvt": np.ascontiguousarray(Wv[hsl].T).astype(bf16),
            "wot": np.ascontiguousarray(Wo[:, hsl].T).astype(bf16),
            "mtd": mtds[b],
            "bq": np.ascontiguousarray((bq[hsl] * scale).reshape(4, 128).T),
            "bk": np.ascontiguousarray(bk[hsl].reshape(4, 128).T),
        })
    return in_maps


def kernel(**inputs):
    in_maps = _host_prep(**inputs)
    nc = _get_prog()
    res = run_bass_kernel_spmd(nc, in_maps, core_ids=list(range(N_CORES)))
    # Constant bias terms pass through the attention average unchanged:
    # out += bo + bv @ Wo^T  (z-normalized ones-column makes bv exact).
    bv = np.asarray(inputs["bv"], np.float32)
    bo = np.asarray(inputs["bo"], np.float32)
    Wo = np.asarray(inputs["Wo"], np.float32)
    const_row = bo + bv @ Wo.T
    out = np.empty((B, S, D), np.float32)
    for b in range(B):
        pt = res.results[2 * b]["out"] + res.results[2 * b + 1]["out"]
        out[b] = pt.T + const_row
    return out
